# revision 1
# baseline (speedup 1.0000x reference)
"""Trainium2 Bass kernel for nn_BruteForceUpdater.

Reference computation:
    xs = x[:, 0, :]                       # [256, 128]
    U  = (xs @ W1.T) @ W2.T               # [256, 8256]
    fw_{i+1} = sigmoid(10*(fw_i + U_i - 0.5))   (serial over batch)
    pred_i = fw2_i @ relu(fw1_i @ x_i)    # fw1 = fw[:8192].reshape(64,128)

Distribution over 8 NeuronCores (no collectives; host sums partials):
  * NFW = 8256 = 64*128 (w1 part of the fast weights) + 64 (w2 part).
  * Core c owns fast-w1 row-tiles h in [8c, 8c+8) (1024 rows of W2) and
    every core also replicates the trailing 64 rows (the fast-w2 part),
    so each core processes a uniform slice of 1088 W2 rows.
  * The host hands each core ONE streaming tensor wcomb [16512, 1216]:
    cols 0:1088 = its W2 slice pre-transposed, cols 1088:1216 = the
    k-block-transposed updater W1 (each 128-row k-tile DMA carries both
    the GEMM weights and the W1 chunk for T1 = W1 @ xs.T).
  * Per core: T1.T tiles are produced on-device two k-tiles ahead of a
    129-tile K-accumulated GEMM into the U.T slice (float32r matmuls,
    psum sub-bank packed), then a 256-step sigmoid scan over the slice,
    then partial predictions sum_{h in core} w2_h*relu(H_h) as [1, 256].
    The host sums the 8 partials.

The streaming loop runs inside a tile_critical block with hand-rolled
semaphores: walrus allows only ~2 sync commands per LDWEIGHTS-matmul /
DMA pseudo-instruction, so each instruction here carries at most one
wait plus one increment, relying on the PE's in-order completion for
transitive coverage.
"""
import os
import sys

sys.path.insert(0, "/opt/trn_rl_repo")

import numpy as np
from contextlib import ExitStack

import concourse.bass as bass
import concourse.tile as tile
from concourse import mybir
from concourse.bass_utils import run_bass_kernel_spmd

F32 = mybir.dt.float32
F32R = mybir.dt.float32r
AF = mybir.ActivationFunctionType

IN = 128
HID = 64
NFW = IN * HID + HID          # 8256
B = 256
K2 = 2 * NFW                  # 16512
KT = K2 // 128                # 129 contraction tiles
NCORES = 8
MT_OWN = 8                    # full 128-row W2 tiles owned per core
NT = MT_OWN + 1               # + shared 64-row tile
MSL = MT_OWN * 128 + HID      # 1088 W2 rows handled per core
WC = MSL + 128                # streamed tile width (W2 cols + W1 chunk)
GAIN, SHIFT = 10.0, 0.5

NSLOT = 16                    # stream ring slots
LOOK = 12                     # DMA lookahead (<= NSLOT - 2)

NT1 = 4                       # t1 ring slots

_NC_CACHE = None


def _build_bass():
    nc = bass.Bass("TRN2", target_bir_lowering=False, debug=False)

    wc_d = nc.dram_tensor("wcomb", [K2, WC], F32R, kind="ExternalInput")
    cst_d = nc.dram_tensor("cst", [128, B + NT + MT_OWN + 1], F32,
                           kind="ExternalInput")
    pred_d = nc.dram_tensor("pred", [1, B], F32, kind="ExternalOutput")
    hs_d = nc.dram_tensor("hscratch", [MT_OWN, B], F32)
    dbg = bool(int(os.environ.get("KDBG", "0")))
    if dbg:
        u_dbg_d = nc.dram_tensor("u_dbg", [128, NT * B], F32,
                                 kind="ExternalOutput")
        fw_dbg_d = nc.dram_tensor("fw_dbg", [128, NT * B], F32,
                                  kind="ExternalOutput")
        t1_dbg_d = nc.dram_tensor("t1_dbg", [128, NT1 * B], F32,
                                  kind="ExternalOutput")
        hsb_dbg_d = nc.dram_tensor("hsb_dbg", [MT_OWN, B], F32,
                                   kind="ExternalOutput")
        w2t_dbg_d = nc.dram_tensor("w2t_dbg", [MT_OWN, B], F32,
                                   kind="ExternalOutput")
        hflat_dbg_d = nc.dram_tensor("hflat_dbg", [1, MT_OWN * B], F32,
                                     kind="ExternalOutput")

    with tile.TileContext(nc) as tc:
        with ExitStack() as ctx:
            const_pool = ctx.enter_context(tc.tile_pool(name="const", bufs=1))
            stream_pool = ctx.enter_context(tc.tile_pool(name="wcs", bufs=1))
            big_pool = ctx.enter_context(tc.tile_pool(name="big", bufs=1))

            cst = const_pool.tile([128, B + NT + MT_OWN + 1], F32)
            xst = cst[:, 0:B]
            fw0_t = cst[:, B:B + NT]
            sel_t = cst[0:HID, B + NT:B + NT + MT_OWN]
            ones_t = cst[:, B + NT + MT_OWN:B + NT + MT_OWN + 1]
            bias_t = const_pool.tile([128, 1], F32)
            zeros64_f = const_pool.tile([128, 128], F32)
            zeros64_r = const_pool.tile([128, 128], F32R)
            xst_r = const_pool.tile([128, B], F32R)

            wbuf = stream_pool.tile([128, NSLOT * WC], F32R)   # stream ring
            t1r = big_pool.tile([128, NT1 * B], F32R)          # T1.T ring
            u_sb = big_pool.tile([128, NT * B], F32)           # U.T slice
            fw_sb = big_pool.tile([128, NT * B], F32)          # fw history
            t_big = big_pool.tile([128, 2 * NT], F32)
            prod_big = big_pool.tile([128, MT_OWN * B], F32)
            w2t_sb = big_pool.tile([MT_OWN, B], F32)
            h_flat = big_pool.tile([1, MT_OWN * B], F32)
            h_sb = big_pool.tile([MT_OWN, B], F32)
            r_sb = big_pool.tile([MT_OWN, B], F32)
            p_sb = big_pool.tile([MT_OWN, B], F32)
            pred_sb = big_pool.tile([1, B], F32)

            u_r = u_sb[:].rearrange("p (m i) -> p m i", m=NT)
            fw_r = fw_sb[:].rearrange("p (m i) -> p m i", m=NT)

            def wslot(j):
                s = j % NSLOT
                return wbuf[:, s * WC:(s + 1) * WC]

            def t1slot(j):
                s = j % NT1
                return t1r[:, s * B:(s + 1) * B]

            # sub-bank psum packing for the 9 accumulating U.T tiles: only
            # the first tile in each 2KB bank starts (start zeroes the whole
            # bank's has_written) and only the last stops.
            tile_bytes = B * 4
            bankof = [m * tile_bytes // 2048 for m in range(NT)]
            m_first = [m == 0 or bankof[m] != bankof[m - 1] for m in range(NT)]
            m_last = [m == NT - 1 or bankof[m] != bankof[m + 1] for m in range(NT)]
            hbank = [m * tile_bytes // 2048 for m in range(MT_OWN)]
            h_first = [m == 0 or hbank[m] != hbank[m - 1] for m in range(MT_OWN)]
            h_last = [m == MT_OWN - 1 or hbank[m] != hbank[m + 1]
                      for m in range(MT_OWN)]

            csem = nc.alloc_semaphore("csem")
            dsem = [nc.alloc_semaphore(f"dsem{s}") for s in range(NSLOT)]
            tmm_sem = nc.alloc_semaphore("tmm")
            cp_sem = nc.alloc_semaphore("cp")
            pe_sem = nc.alloc_semaphore("pe")
            sv = nc.alloc_semaphore("sv")     # DVE progress
            sa = nc.alloc_semaphore("sa")     # ACT progress
            pp = nc.alloc_semaphore("pp")     # PE pred progress
            dsm = nc.alloc_semaphore("dsm")   # pred-phase DMA

            with tc.tile_pool(name="pt1", bufs=1, space="PSUM") as pt_pool, \
                 tc.tile_pool(name="pu", bufs=1, space="PSUM") as pu_pool:
                # pt slots are bank-aligned (512 f32 apart) so each T1
                # matmul's start=True only clears its own bank
                pt = pt_pool.tile([128, 1024], F32)
                psum_u = pu_pool.tile([128, NT * B], F32)
                # pred-phase psum aliases dead GEMM psum regions
                psum_w2t = pt[0:MT_OWN, 0:B]
                psum_h = psum_u[0:1, 0:MT_OWN * B]
                psum_p = pt[0:1, 512:512 + B]

                with tc.tile_critical():
                    svc = [0]                 # sv value tracker

                    def dve_inc(inst):
                        inst.then_inc(sv, 1)
                        svc[0] += 1
                        return svc[0]

                    # constants: one DMA, then DVE preps
                    nc.gpsimd.dma_start(cst[:], cst_d[:, :]).then_inc(csem, 16)
                    nc.vector.memset(bias_t[:], -GAIN * SHIFT)
                    msz = nc.vector.memset(zeros64_f[:], 0.0)
                    v_msz = dve_inc(msz)
                    zc = nc.vector.tensor_copy(zeros64_r[:], zeros64_f[:])
                    zc._wait_ge(sv, v_msz)
                    dve_inc(zc)
                    cxr = nc.vector.tensor_copy(xst_r[:], xst)
                    cxr._wait_ge(csem, 16)
                    v_xr = dve_inc(cxr)       # sv: memsets+consts ready

                    def dma_k(j):
                        d = nc.sync.dma_start(
                            wslot(j), wc_d[j * 128:(j + 1) * 128, :])
                        if j >= NSLOT:
                            # slot free once GEMM2(j-NSLOT) fully read it
                            d._wait_ge(pe_sem, j - NSLOT + 1)
                        d.then_inc(dsem[j % NSLOT], 16)

                    def t1_mm(j):
                        mm = nc.tensor.matmul(
                            pt[:, (j % 2) * 512:(j % 2) * 512 + B],
                            wslot(j)[:, MSL:WC], xst_r[:],
                            start=True, stop=True)
                        mm._wait_ge(dsem[j % NSLOT], 16 * (j // NSLOT + 1))
                        mm.then_inc(tmm_sem, 1)

                    def t1_copy(j):
                        cp = nc.vector.tensor_copy(
                            t1slot(j), pt[:, (j % 2) * 512:(j % 2) * 512 + B])
                        cp._wait_ge(tmm_sem, j + 1)
                        cp.then_inc(cp_sem, 1)

                    # PE warmup into pt slot 0: pulls the const/DVE prep
                    # tick into the PE's clock (T1mm(0) overwrites it next)
                    zmm = nc.tensor.matmul(pt[:, 0:B],
                                           zeros64_r[:], xst_r[:],
                                           start=True, stop=True)
                    zmm._wait_ge(sv, v_xr)

                    for j in range(LOOK):
                        dma_k(j)
                    for j in range(2):
                        t1_mm(j)
                        t1_copy(j)

                    for k in range(KT):
                        if k + LOOK < KT:
                            dma_k(k + LOOK)
                        for m in range(NT):
                            mm = nc.tensor.matmul(
                                psum_u[:, m * B:(m + 1) * B],
                                wslot(k)[:, m * 128:(m + 1) * 128],
                                t1slot(k),
                                start=(k == 0 and m_first[m]),
                                stop=(k == KT - 1 and m_last[m]),
                            )
                            if m == 0:
                                mm._wait_ge(cp_sem, k + 1)
                            if m == NT - 1:
                                mm.then_inc(pe_sem, 1)
                        if k + 2 < KT:
                            t1_mm(k + 2)
                            t1_copy(k + 2)

                    cpu = nc.vector.tensor_copy(u_sb[:], psum_u[:])
                    cpu._wait_ge(pe_sem, KT)
                    v_ucp = dve_inc(cpu)

                    # ---- 256-step sigmoid scan ----
                    sa_base = 0
                    for i in range(B):
                        t_t = t_big[:, (i % 2) * NT:(i % 2) * NT + NT]
                        prev = fw0_t if i == 0 else fw_r[:, :, i - 1]
                        add = nc.vector.tensor_add(t_t, prev, u_r[:, :, i])
                        if i > 0:
                            add._wait_ge(sa, i)
                        else:
                            add._wait_ge(sv, v_ucp)
                        v_add = dve_inc(add)
                        act = nc.scalar.activation(
                            fw_r[:, :, i], t_t, AF.Sigmoid,
                            bias=bias_t[:], scale=GAIN)
                        act._wait_ge(sv, v_add)
                        act.then_inc(sa, 1)

                    # ---- partial predictions ----
                    v_prod = []
                    for m in range(MT_OWN):
                        pr = nc.vector.tensor_mul(
                            prod_big[:, m * B:(m + 1) * B],
                            fw_r[:, m, :], xst)
                        if m == 0:
                            pr._wait_ge(sa, B)
                        v_prod.append(dve_inc(pr))
                    sel_mm = nc.tensor.matmul(
                        psum_w2t[:], sel_t, fw_r[0:HID, NT - 1, :],
                        start=True, stop=True)
                    sel_mm._wait_ge(sv, v_prod[0])  # implies sa >= B
                    for m in range(MT_OWN):
                        hm = nc.tensor.matmul(
                            psum_h[0:1, m * B:(m + 1) * B], ones_t,
                            prod_big[:, m * B:(m + 1) * B],
                            start=h_first[m], stop=h_last[m])
                        hm._wait_ge(sv, v_prod[m])
                        if m == MT_OWN - 1:
                            hm.then_inc(pp, 1)
                    cp1 = nc.vector.tensor_copy(w2t_sb[:], psum_w2t[:])
                    cp1._wait_ge(pp, 1)
                    dve_inc(cp1)
                    cp2 = nc.vector.tensor_copy(h_flat[:], psum_h[:])
                    v_hflat = dve_inc(cp2)
                    dh1 = nc.sync.dma_start(hs_d[:, :], h_flat[0:1, :])
                    dh1._wait_ge(sv, v_hflat)
                    dh1.then_inc(dsm, 16)
                    dh2 = nc.sync.dma_start(h_sb[:], hs_d[:, :])
                    dh2._wait_ge(dsm, 16)
                    dh2.then_inc(dsm, 16)
                    rl = nc.vector.tensor_relu(r_sb[:], h_sb[:])
                    rl._wait_ge(dsm, 32)
                    v_relu = dve_inc(rl)
                    pm = nc.vector.tensor_mul(p_sb[:], r_sb[:], w2t_sb[:])
                    pm._wait_ge(sv, v_relu)
                    v_psb = dve_inc(pm)
                    pmm = nc.tensor.matmul(psum_p[:], ones_t[0:MT_OWN, :],
                                           p_sb[:], start=True, stop=True)
                    pmm._wait_ge(sv, v_psb)
                    pmm.then_inc(pp, 1)
                    cp3 = nc.vector.tensor_copy(pred_sb[:], psum_p[:])
                    cp3._wait_ge(pp, 2)
                    v_pred = dve_inc(cp3)
                    dout = nc.sync.dma_start(pred_d[:, :], pred_sb[:])
                    dout._wait_ge(sv, v_pred)
                    dout.then_inc(dsm, 16)
                    if dbg:
                        du = nc.sync.dma_start(u_dbg_d[:, :], u_sb[:])
                        du._wait_ge(sv, v_pred)
                        du.then_inc(dsm, 16)
                        df = nc.sync.dma_start(fw_dbg_d[:, :], fw_sb[:])
                        df._wait_ge(sv, v_pred)
                        df.then_inc(dsm, 16)
                        dt1 = nc.sync.dma_start(
                            t1_dbg_d[:, :], t1r[:].bitcast(F32))
                        dt1._wait_ge(sv, v_pred)
                        dt1.then_inc(dsm, 16)
                        for dd, ss in ((hsb_dbg_d, h_sb), (w2t_dbg_d, w2t_sb),
                                       (hflat_dbg_d, h_flat)):
                            dx = nc.sync.dma_start(dd[:, :], ss[:])
                            dx._wait_ge(sv, v_pred)
                            dx.then_inc(dsm, 16)

    _dedupe_waits(nc)
    return nc


def _dedupe_waits(nc):
    """Collapse duplicate semaphore waits the framework occasionally emits
    (e.g. critical-entry branches) — walrus allows very few sync commands
    per instruction."""
    for fnn in nc.m.functions:
        for blk in fnn.blocks:
            for inst in blk.instructions:
                si = inst.sync_info
                if si is None or not si.on_wait or len(si.on_wait) < 2:
                    continue
                best = {}
                order = []
                for w in si.on_wait:
                    if w.wait_reg is not None or w.wait_mode != "sem-ge-imm":
                        key = ("raw", id(w))
                    else:
                        key = (w.sync_type, w.id, w.wait_mode)
                    if key not in best:
                        best[key] = w
                        order.append(key)
                    elif (w.wait_value or 0) > (best[key].wait_value or 0):
                        best[key] = w
                deduped = [best[k] for k in order]
                if len(deduped) != len(si.on_wait):
                    inst.sync_info = mybir.SyncInfo(
                        on_wait=deduped, on_update=si.on_update)
def _split_noops(nc):
    """Split multi-wait NoOps into single-wait chains (walrus's CTRL_NO
    struct carries very few sync commands). Applied lazily before HW runs
    only — CoreSim rejects instructions without its fake-update records."""
    if getattr(nc, "_noops_split", False):
        return
    nc._noops_split = True
    split_id = [0]
    for fnn in nc.m.functions:
        for blk in fnn.blocks:
            out = []
            changed = False
            for inst in blk.instructions:
                si = inst.sync_info
                if (type(inst).__name__ == "InstNoOp" and si is not None
                        and len(si.on_wait) > 1):
                    changed = True
                    for w in si.on_wait[:-1]:
                        no = mybir.InstNoOp(
                            name=f"noop_waitsplit_{split_id[0]}",
                            text_hint="waitsplit")
                        split_id[0] += 1
                        no.engine = inst.engine
                        no.sync_info = mybir.SyncInfo(
                            on_wait=[w], on_update=[])
                        out.append(no)
                    inst.sync_info = mybir.SyncInfo(
                        on_wait=[si.on_wait[-1]], on_update=si.on_update)
                out.append(inst)
            if changed:
                blk.instructions = out


def _get_nc():
    global _NC_CACHE
    if _NC_CACHE is None:
        _NC_CACHE = _build_bass()
    return _NC_CACHE


def _make_in_maps(x, W1, W2, fw0):
    xs = np.ascontiguousarray(x[:, 0, :].astype(np.float32))       # [256, 128]
    xst = np.ascontiguousarray(xs.T)                                # [128, 256]
    W1 = np.asarray(W1, dtype=np.float32)
    W2 = np.asarray(W2, dtype=np.float32)
    fw0 = np.asarray(fw0, dtype=np.float32)
    ones = np.ones((128, 1), np.float32)

    # k-block-transposed W1: rows k*128+p, col c = W1[k*128+c, p]
    w1bt = np.ascontiguousarray(
        W1.reshape(KT, 128, IN).transpose(0, 2, 1).reshape(K2, 128))

    shared_rows = W2[MT_OWN * 128 * NCORES:, :]                     # [64, 16512]
    fw_shared = np.zeros(128, np.float32)
    fw_shared[0:HID] = fw0[MT_OWN * 128 * NCORES:]

    in_maps = []
    for c in range(NCORES):
        own = W2[c * 1024:(c + 1) * 1024, :]                        # [1024, 16512]
        w2c = np.concatenate([own, shared_rows], axis=0)            # [1088, 16512]
        wcomb = np.concatenate(
            [np.ascontiguousarray(w2c.T), w1bt], axis=1)            # [16512, 1216]
        fw0_t = np.zeros((128, NT), np.float32)
        for m in range(MT_OWN):
            fw0_t[:, m] = fw0[c * 1024 + m * 128: c * 1024 + (m + 1) * 128]
        fw0_t[:, NT - 1] = fw_shared
        sel = np.zeros((HID, MT_OWN), np.float32)
        for m in range(MT_OWN):
            sel[MT_OWN * c + m, m] = 1.0
        cst = np.zeros((128, B + NT + MT_OWN + 1), np.float32)
        cst[:, 0:B] = xst
        cst[:, B:B + NT] = fw0_t
        cst[0:HID, B + NT:B + NT + MT_OWN] = sel
        cst[:, B + NT + MT_OWN] = 1.0
        in_maps.append({
            "wcomb": np.ascontiguousarray(wcomb),
            "cst": cst,
        })
    return in_maps


def kernel(x, W1, W2, fw0, _trace=False, _tmpdir=None):
    nc = _get_nc()
    _split_noops(nc)
    in_maps = _make_in_maps(x, W1, W2, fw0)
    res = run_bass_kernel_spmd(
        nc, in_maps, core_ids=list(range(NCORES)),
        trace=_trace, tmpdir=_tmpdir,
    )
    preds = np.zeros((1, B), np.float64)
    for c in range(NCORES):
        preds += res.results[c]["pred"].astype(np.float64)
    out = preds.astype(np.float32).reshape(B, 1)
    if _trace:
        return out, res
    return out



# revision 8
# speedup vs baseline: 1.6924x; 1.6924x over previous
"""Trainium2 Bass kernel for nn_BruteForceUpdater (fp16 GEMM + chunked scan).

Reference computation:
    xs = x[:, 0, :]                       # [256, 128]
    U  = (xs @ W1.T) @ W2.T               # [256, 8256]
    fw_{i+1} = sigmoid(10*(fw_i + U_i - 0.5))   (serial over batch)
    pred_i = fw2_i @ relu(fw1_i @ x_i)    # fw1 = fw[:8192].reshape(64,128)

Distribution over 8 NeuronCores (no collectives; host sums partials):
  * Core c owns fast-w1 row-tiles h in [8c, 8c+8) (1024 rows of W2); every
    core replicates the trailing 64 rows (the fast-w2 part) -> each core
    processes 1088 W2 rows = 9 output tiles (NT) of U.T.
  * Streamed tensor wq: the k-blocked weights, fp16, packed 3 k-tiles per
    DMA row-block ([128, 3*1216]): cols 0:1088 of each k-tile = the W2
    slice pre-transposed, cols 1088:1216 = the k-block-transposed W1 chunk
    (produces T1 = W1 @ xs.T on device, two k-tiles ahead of the GEMM).
  * GEMM: fp16 operands (error ~5e-4 << the 2e-2 gate), psum fp32
    accumulation over 129 k-tiles into U.T [9 tiles x 256].
  * Scan: the sigmoid recurrence contracts hard (gain 10 saturates), so
    the 256-step chain is split into 4 chunks of 64 steps; chunks 1..3
    rerun H=[18,21,24] warmup steps from state 0.5 (numpy-validated to
    reproduce the exact scan to ~5e-4).  The 4 chains interleave
    round-robin on DVE (add) + ACT (sigmoid), hiding the cross-engine
    latency that made the serial scan 552 ns/step.
  * Prediction partials per chunk are computed as soon as a chunk's
    columns finish, on the otherwise-idle Pool engine + PE, so only the
    last chunk's tail (~4 us) follows the scan.

Hand-rolled semaphores (one wait + one inc per instruction, walrus
limit); in-order engine queues carry the rest of the ordering.
"""
import os
import sys

sys.path.insert(0, "/opt/trn_rl_repo")

import numpy as np
from contextlib import ExitStack

import concourse.bass as bass
import concourse.tile as tile
from concourse import mybir
from concourse.bass_utils import run_bass_kernel_spmd

F32 = mybir.dt.float32
F16 = mybir.dt.float16
AF = mybir.ActivationFunctionType
ALU = mybir.AluOpType
AX = mybir.AxisListType

IN = 128
HID = 64
NFW = IN * HID + HID          # 8256
B = 256
K2 = 2 * NFW                  # 16512
KT = K2 // 128                # 129 contraction tiles
NCORES = 8
MT_OWN = 8                    # full 128-row W2 tiles owned per core
NT = MT_OWN + 1               # + shared 64-row tile
MSL = MT_OWN * 128 + HID      # 1088 W2 rows handled per core
WC = MSL + 128                # k-tile width (W2 cols + W1 chunk)
GAIN, SHIFT = 10.0, 0.5

T3 = 3                        # k-tiles per streamed DMA
QT = KT // T3                 # 43 triple-DMAs
NQ = 8                        # triple ring slots
LOOKQ = 7                     # triples issued ahead
NT1 = 4                       # t1 ring slots

CH = 4                        # scan chunks
LCH = B // CH                 # 64 columns per chunk
HWRM = [0, 18, 21, 24]        # warmup steps per chunk
ENDS = [HWRM[c] + LCH for c in range(CH)]   # [64, 82, 85, 88]
ROUNDS = max(ENDS)

_NC_CACHE = None


def _build_bass():
    nc = bass.Bass("TRN2", target_bir_lowering=False, debug=False)

    wq_d = nc.dram_tensor("wq", [QT * 128, T3 * WC], F16, kind="ExternalInput")
    cst_d = nc.dram_tensor("cst", [128, B + NT], F32, kind="ExternalInput")
    c16_d = nc.dram_tensor("c16", [128, 9], F16, kind="ExternalInput")
    pred_d = nc.dram_tensor("pred", [1, B], F32, kind="ExternalOutput")
    dbg = bool(int(os.environ.get("KDBG", "0")))
    if dbg:
        u_dbg_d = nc.dram_tensor("u_dbg", [128, NT * B], F32,
                                 kind="ExternalOutput")
        fw_dbg_d = nc.dram_tensor("fw_dbg", [128, NT * B], F32,
                                  kind="ExternalOutput")
        q_dbg_d = nc.dram_tensor("q_dbg", [1, CH * 512], F32,
                                 kind="ExternalOutput")

    with tile.TileContext(nc) as tc:
        with ExitStack() as ctx:
            const_pool = ctx.enter_context(tc.tile_pool(name="const", bufs=1))
            stream_pool = ctx.enter_context(tc.tile_pool(name="wcs", bufs=1))
            big_pool = ctx.enter_context(tc.tile_pool(name="big", bufs=1))

            cst = const_pool.tile([128, B + NT], F32)
            xst = cst[:, 0:B]
            fw0_t = cst[:, B:B + NT]
            c16 = const_pool.tile([128, 9], F16)
            xst_h = const_pool.tile([128, B], F16)
            bias_t = const_pool.tile([128, 1], F32)
            half_t = const_pool.tile([128, NT], F32)
            zf16 = const_pool.tile([128, 128], F16)

            wbuf = stream_pool.tile([128, NQ * T3 * WC], F16)  # stream ring
            t1r = big_pool.tile([128, NT1 * B], F16)           # T1.T ring
            u_sb = big_pool.tile([128, NT * B], F32)           # U.T slice
            fw_sb = big_pool.tile([128, NT * B], F32)          # fw history
            t_big = big_pool.tile([128, CH * 2 * NT], F32)     # add ping-pong
            wsc = big_pool.tile([128, CH * 2 * NT], F32)       # warmup states
            prod_h = big_pool.tile([128, MT_OWN * B], F16)     # fw1*x products
            w2h = big_pool.tile([HID, B], F16)                 # fw2 cast
            rf = big_pool.tile([1, CH * 512], F32)             # relu(H) flat
            w2f = big_pool.tile([1, CH * 512], F32)            # w2 flat
            qf = big_pool.tile([1, CH * 512], F32)             # products flat
            pred_sb = big_pool.tile([1, B], F32)

            u_r = u_sb[:].rearrange("p (m i) -> p m i", m=NT)
            fw_r = fw_sb[:].rearrange("p (m i) -> p m i", m=NT)

            def qslot(q):
                s = q % NQ
                return wbuf[:, s * T3 * WC:(s + 1) * T3 * WC]

            def wslot(k):
                s = (k // T3) % NQ
                off = (s * T3 + k % T3) * WC
                return wbuf[:, off:off + WC]

            def t1slot(j):
                s = j % NT1
                return t1r[:, s * B:(s + 1) * B]

            # sub-bank psum packing for the 9 accumulating U.T tiles: only
            # the first tile in each 2KB bank starts (start zeroes the whole
            # bank) and only the last stops.
            tile_bytes = B * 4
            bankof = [m * tile_bytes // 2048 for m in range(NT)]
            m_first = [m == 0 or bankof[m] != bankof[m - 1] for m in range(NT)]
            m_last = [m == NT - 1 or bankof[m] != bankof[m + 1] for m in range(NT)]

            csem = nc.alloc_semaphore("csem")
            dsem = [nc.alloc_semaphore(f"dsem{s}") for s in range(NQ)]
            tmm_sem = nc.alloc_semaphore("tmm")
            cp_sem = nc.alloc_semaphore("cp")
            pe_sem = nc.alloc_semaphore("pe")
            sv = nc.alloc_semaphore("sv")     # DVE progress
            sa = nc.alloc_semaphore("sa")     # ACT progress
            pl = nc.alloc_semaphore("pl")     # Pool progress
            pp = nc.alloc_semaphore("pp")     # PE pred progress
            dsm = nc.alloc_semaphore("dsm")

            with tc.tile_pool(name="pt1", bufs=1, space="PSUM") as pt_pool, \
                 tc.tile_pool(name="pu", bufs=1, space="PSUM") as pu_pool:
                # pt slots are bank-aligned (512 f32) for T1 ping-pong;
                # after the GEMM the two banks hold the per-chunk w2 flats
                pt = pt_pool.tile([128, 1024], F32)
                psum_u = pu_pool.tile([128, NT * B], F32)
                pu_r = psum_u[:].rearrange("p (m i) -> p m i", m=NT)

                with tc.tile_critical():
                    svc = [0]                 # sv value tracker

                    def dve_inc(inst):
                        inst.then_inc(sv, 1)
                        svc[0] += 1
                        return svc[0]

                    # constants: two DMAs on the Pool queue, DVE preps
                    nc.gpsimd.dma_start(cst[:], cst_d[:, :]).then_inc(csem, 16)
                    nc.gpsimd.dma_start(c16[:], c16_d[:, :]).then_inc(csem, 16)
                    nc.vector.memset(bias_t[:], -GAIN * SHIFT)
                    dve_inc(nc.vector.memset(half_t[:], 0.5))
                    dve_inc(nc.vector.memset(zf16[:], 0.0))
                    cxr = nc.vector.tensor_copy(xst_h[:], xst)
                    cxr._wait_ge(csem, 32)
                    v_xr = dve_inc(cxr)       # consts + casts ready

                    def dma_q(q):
                        d = nc.sync.dma_start(
                            qslot(q), wq_d[q * 128:(q + 1) * 128, :])
                        if q >= NQ:
                            # slot free once GEMM consumed all 3 k-tiles
                            d._wait_ge(pe_sem, T3 * (q - NQ) + T3)
                        d.then_inc(dsem[q % NQ], 16)

                    def t1_mm(j):
                        mm = nc.tensor.matmul(
                            pt[:, (j % 2) * 512:(j % 2) * 512 + B],
                            wslot(j)[:, MSL:WC], xst_h[:],
                            start=True, stop=True)
                        q = j // T3
                        mm._wait_ge(dsem[q % NQ], 16 * (q // NQ + 1))
                        mm.then_inc(tmm_sem, 1)

                    def t1_copy(j):
                        cp = nc.vector.tensor_copy(
                            t1slot(j), pt[:, (j % 2) * 512:(j % 2) * 512 + B])
                        cp._wait_ge(tmm_sem, j + 1)
                        cp.then_inc(cp_sem, 1)

                    # PE warmup (pulls the const prep tick into PE's clock)
                    zmm = nc.tensor.matmul(pt[:, 0:B], zf16[:], xst_h[:],
                                           start=True, stop=True)
                    zmm._wait_ge(sv, v_xr)

                    for q in range(LOOKQ):
                        dma_q(q)
                    for j in range(2):
                        t1_mm(j)
                        t1_copy(j)

                    for k in range(KT):
                        if k % T3 == 0 and k // T3 + LOOKQ < QT:
                            dma_q(k // T3 + LOOKQ)
                        for m in range(NT):
                            mm = nc.tensor.matmul(
                                psum_u[:, m * B:(m + 1) * B],
                                wslot(k)[:, m * 128:(m + 1) * 128],
                                t1slot(k),
                                start=(k == 0 and m_first[m]),
                                stop=(k == KT - 1 and m_last[m]),
                            )
                            if m == 0:
                                mm._wait_ge(cp_sem, k + 1)
                            if m == NT - 1:
                                mm.then_inc(pe_sem, 1)
                        if k + 2 < KT:
                            t1_mm(k + 2)
                            t1_copy(k + 2)

                    # ---- chunked sigmoid scan ----
                    # chunk c covers columns [c*LCH, (c+1)*LCH); chunks 1..3
                    # rerun HWRM[c] warmup steps from state 0.5 first.
                    def col_of(c, r):
                        return c * LCH - HWRM[c] + r

                    def tslot_of(c, r):
                        off = (2 * c + r % 2) * NT
                        return t_big[:, off:off + NT]

                    def wslot_of(c, r):
                        off = (2 * c + r % 2) * NT
                        return wsc[:, off:off + NT]

                    v_add = {}
                    a_idx = {}
                    a_cnt = [0]

                    def emit_add(c, r):
                        i = col_of(c, r)
                        if r == 0:
                            prev = fw0_t if c == 0 else half_t
                        elif r - 1 < HWRM[c]:
                            prev = wslot_of(c, r - 1)
                        else:
                            prev = fw_r[:, :, i - 1]
                        add = nc.vector.tensor_add(
                            tslot_of(c, r), prev, u_r[:, :, i])
                        if r > 0:
                            add._wait_ge(sa, a_idx[(c, r - 1)])
                        v_add[(c, r)] = dve_inc(add)

                    def emit_act(c, r):
                        i = col_of(c, r)
                        out = wslot_of(c, r) if r < HWRM[c] else fw_r[:, :, i]
                        act = nc.scalar.activation(
                            out, tslot_of(c, r), AF.Sigmoid,
                            bias=bias_t[:], scale=GAIN)
                        act._wait_ge(sv, v_add[(c, r)])
                        act.then_inc(sa, 1)
                        a_cnt[0] += 1
                        a_idx[(c, r)] = a_cnt[0]

                    # round 0 interleaved with the psum->sbuf U copies
                    for c in range(CH):
                        ucp = nc.vector.tensor_copy(
                            u_r[:, :, c * LCH:(c + 1) * LCH],
                            pu_r[:, :, c * LCH:(c + 1) * LCH])
                        if c == 0:
                            ucp._wait_ge(pe_sem, KT)
                        dve_inc(ucp)
                        emit_add(c, 0)
                        emit_act(c, 0)

                    # Pool op ordering (GPSIMD cannot touch PSUM, so the
                    # psum->sbuf relu/copy-outs live on DVE, injected into
                    # the scan stream; Pool does the sbuf-only work):
                    #   [p0 c0 mul0 | p1 c1 | p2 c2 mul1 | p3 c3 mul2 mul3]
                    # pl counts:  9,10 | 19 | 28,29 | 38,39,40
                    PL_PC = [9, 19, 28, 38]    # prods+cast done per chunk
                    PL_MUL = [10, 29, 39, 40]  # mul done per chunk
                    v_red = [None] * CH
                    v_w2cp = [None] * CH

                    def emit_pool_pc(c):
                        cols = slice(c * LCH, (c + 1) * LCH)
                        for m in range(MT_OWN):
                            pr = nc.gpsimd.tensor_mul(
                                prod_h[:, m * B + c * LCH:m * B + (c + 1) * LCH],
                                fw_r[:, m, cols], xst[:, cols])
                            if m == 0:
                                pr._wait_ge(sa, a_idx[(c, ENDS[c] - 1)])
                            pr.then_inc(pl, 1)
                        cw = nc.gpsimd.tensor_copy(
                            w2h[:, cols], fw_r[0:HID, NT - 1, cols])
                        cw.then_inc(pl, 1)

                    def emit_pool_mul(c):
                        ml = nc.gpsimd.tensor_mul(
                            qf[0:1, c * 512:c * 512 + 512],
                            rf[0:1, c * 512:c * 512 + 512],
                            w2f[0:1, c * 512:c * 512 + 512])
                        ml._wait_ge(sv, v_w2cp[c])
                        ml.then_inc(pl, 1)

                    def emit_relu_w2cp(c):
                        rl = nc.vector.tensor_relu(
                            rf[0:1, c * 512:c * 512 + 512],
                            psum_u[0:1, (c % 2) * 512:(c % 2) * 512 + 512])
                        rl._wait_ge(pp, 2 * c + 1)
                        dve_inc(rl)
                        wcp = nc.vector.tensor_copy(
                            w2f[0:1, c * 512:c * 512 + 512],
                            pt[0:1, (c % 2) * 512:(c % 2) * 512 + 512])
                        wcp._wait_ge(pp, 2 * c + 2)
                        v_w2cp[c] = dve_inc(wcp)

                    def emit_reduce(c):
                        red = nc.vector.tensor_reduce(
                            pred_sb[0:1, c * LCH:(c + 1) * LCH],
                            qf[0:1, c * 512:c * 512 + 512].rearrange(
                                "p (m b) -> p b m", m=MT_OWN),
                            axis=AX.X, op=ALU.add)
                        red._wait_ge(pl, PL_MUL[c])
                        v_red[c] = dve_inc(red)

                    for r in range(1, ROUNDS):
                        for c in range(CH):
                            if r < ENDS[c]:
                                emit_add(c, r)
                                emit_act(c, r)
                        if r == 70:
                            emit_relu_w2cp(0)    # chunk 0 tail, hidden
                        elif r == 74:
                            emit_reduce(0)
                        elif r == 86:
                            emit_relu_w2cp(1)

                    # ---- post-scan DVE tail (reduce1 fills the PE/Pool gap) ----
                    emit_reduce(1)
                    emit_relu_w2cp(2)
                    emit_relu_w2cp(3)
                    emit_reduce(2)
                    emit_reduce(3)

                    # ---- Pool program (sbuf-only prediction work) ----
                    emit_pool_pc(0)
                    emit_pool_mul(0)
                    emit_pool_pc(1)
                    emit_pool_pc(2)
                    emit_pool_mul(1)
                    emit_pool_pc(3)
                    emit_pool_mul(2)
                    emit_pool_mul(3)

                    # PE column sums: H[m,b] and w2 extraction, flat on
                    # psum partition 0 (chunk-major 512-f32 = one bank each)
                    for c in range(CH):
                        cols = slice(c * LCH, (c + 1) * LCH)
                        hoff = (c % 2) * 512
                        for m in range(MT_OWN):
                            hm = nc.tensor.matmul(
                                psum_u[0:1, hoff + m * LCH:hoff + (m + 1) * LCH],
                                c16[:, 0:1],
                                prod_h[:, m * B + c * LCH:m * B + (c + 1) * LCH],
                                start=(m == 0), stop=(m == MT_OWN - 1))
                            if m == 0:
                                hm._wait_ge(pl, PL_PC[c])
                            if m == MT_OWN - 1:
                                hm.then_inc(pp, 1)
                        for s in range(MT_OWN):
                            wm = nc.tensor.matmul(
                                pt[0:1, hoff + s * LCH:hoff + (s + 1) * LCH],
                                c16[0:HID, 1 + s:2 + s], w2h[:, cols],
                                start=(s == 0), stop=(s == MT_OWN - 1))
                            if s == MT_OWN - 1:
                                wm.then_inc(pp, 1)

                    dout = nc.sync.dma_start(pred_d[:, :], pred_sb[:])
                    dout._wait_ge(sv, v_red[CH - 1])
                    dout.then_inc(dsm, 16)
                    if dbg:
                        du = nc.sync.dma_start(u_dbg_d[:, :], u_sb[:])
                        du._wait_ge(sv, v_red[CH - 1])
                        du.then_inc(dsm, 16)
                        df = nc.sync.dma_start(fw_dbg_d[:, :], fw_sb[:])
                        df._wait_ge(sv, v_red[CH - 1])
                        df.then_inc(dsm, 16)
                        dq = nc.sync.dma_start(q_dbg_d[:, :], qf[:])
                        dq._wait_ge(sv, v_red[CH - 1])
                        dq.then_inc(dsm, 16)

    _dedupe_waits(nc)
    return nc


def _dedupe_waits(nc):
    """Collapse duplicate semaphore waits the framework occasionally emits
    (walrus allows very few sync commands per instruction)."""
    for fnn in nc.m.functions:
        for blk in fnn.blocks:
            for inst in blk.instructions:
                si = inst.sync_info
                if si is None or not si.on_wait or len(si.on_wait) < 2:
                    continue
                best = {}
                order = []
                for w in si.on_wait:
                    if w.wait_reg is not None or w.wait_mode != "sem-ge-imm":
                        key = ("raw", id(w))
                    else:
                        key = (w.sync_type, w.id, w.wait_mode)
                    if key not in best:
                        best[key] = w
                        order.append(key)
                    elif (w.wait_value or 0) > (best[key].wait_value or 0):
                        best[key] = w
                deduped = [best[k] for k in order]
                if len(deduped) != len(si.on_wait):
                    inst.sync_info = mybir.SyncInfo(
                        on_wait=deduped, on_update=si.on_update)


def _split_noops(nc):
    """Split multi-wait NoOps into single-wait chains (walrus's CTRL_NO
    struct carries very few sync commands)."""
    if getattr(nc, "_noops_split", False):
        return
    nc._noops_split = True
    split_id = [0]
    for fnn in nc.m.functions:
        for blk in fnn.blocks:
            out = []
            changed = False
            for inst in blk.instructions:
                si = inst.sync_info
                if (type(inst).__name__ == "InstNoOp" and si is not None
                        and len(si.on_wait) > 1):
                    changed = True
                    for w in si.on_wait[:-1]:
                        no = mybir.InstNoOp(
                            name=f"noop_waitsplit_{split_id[0]}",
                            text_hint="waitsplit")
                        split_id[0] += 1
                        no.engine = inst.engine
                        no.sync_info = mybir.SyncInfo(
                            on_wait=[w], on_update=[])
                        out.append(no)
                    inst.sync_info = mybir.SyncInfo(
                        on_wait=[si.on_wait[-1]], on_update=si.on_update)
                out.append(inst)
            if changed:
                blk.instructions = out


def _get_nc():
    global _NC_CACHE
    if _NC_CACHE is None:
        _NC_CACHE = _build_bass()
    return _NC_CACHE


def _make_in_maps(x, W1, W2, fw0):
    xs = np.ascontiguousarray(x[:, 0, :].astype(np.float32))       # [256, 128]
    xst = np.ascontiguousarray(xs.T)                                # [128, 256]
    W1 = np.asarray(W1, dtype=np.float32)
    W2 = np.asarray(W2, dtype=np.float32)
    fw0 = np.asarray(fw0, dtype=np.float32)

    # k-block-transposed W1: rows k*128+p, col c = W1[k*128+c, p]
    w1bt = np.ascontiguousarray(
        W1.reshape(KT, 128, IN).transpose(0, 2, 1).reshape(K2, 128))

    shared_rows = W2[MT_OWN * 128 * NCORES:, :]                     # [64, 16512]
    fw_shared = np.zeros(128, np.float32)
    fw_shared[0:HID] = fw0[MT_OWN * 128 * NCORES:]

    in_maps = []
    for c in range(NCORES):
        own = W2[c * 1024:(c + 1) * 1024, :]                        # [1024, 16512]
        w2c = np.concatenate([own, shared_rows], axis=0)            # [1088, 16512]
        wcomb = np.concatenate(
            [np.ascontiguousarray(w2c.T), w1bt], axis=1)            # [16512, 1216]
        # pack 3 k-tiles per DMA row-block, fp16
        wq = np.ascontiguousarray(
            wcomb.reshape(QT, T3, 128, WC).transpose(0, 2, 1, 3)
            .reshape(QT * 128, T3 * WC)).astype(np.float16)
        fw0_t = np.zeros((128, NT), np.float32)
        for m in range(MT_OWN):
            fw0_t[:, m] = fw0[c * 1024 + m * 128: c * 1024 + (m + 1) * 128]
        fw0_t[:, NT - 1] = fw_shared
        cst = np.zeros((128, B + NT), np.float32)
        cst[:, 0:B] = xst
        cst[:, B:B + NT] = fw0_t
        c16 = np.zeros((128, 9), np.float16)
        c16[:, 0] = 1.0
        for s in range(MT_OWN):
            c16[MT_OWN * c + s, 1 + s] = 1.0
        in_maps.append({"wq": wq, "cst": cst, "c16": c16})
    return in_maps


def kernel(x, W1, W2, fw0, _trace=False, _tmpdir=None):
    nc = _get_nc()
    _split_noops(nc)
    in_maps = _make_in_maps(x, W1, W2, fw0)
    res = run_bass_kernel_spmd(
        nc, in_maps, core_ids=list(range(NCORES)),
        trace=_trace, tmpdir=_tmpdir,
    )
    preds = np.zeros((1, B), np.float64)
    for c in range(NCORES):
        preds += res.results[c]["pred"].astype(np.float64)
    out = preds.astype(np.float32).reshape(B, 1)
    if _trace:
        return out, res
    return out


# revision 14
# speedup vs baseline: 2.0324x; 1.2009x over previous
"""Trainium2 Bass kernel for nn_BruteForceUpdater (fp16 GEMM + stream scan).

Reference computation:
    xs = x[:, 0, :]                       # [256, 128]
    U  = (xs @ W1.T) @ W2.T               # [256, 8256]
    fw_{i+1} = sigmoid(10*(fw_i + U_i - 0.5))   (serial over batch)
    pred_i = fw2_i @ relu(fw1_i @ x_i)    # fw1 = fw[:8192].reshape(64,128)

Distribution over 8 NeuronCores (no collectives; host sums partials):
  * Core c owns fast-w1 row-tiles h in [8c, 8c+8) (1024 rows of W2); every
    core replicates the trailing 64 rows (the fast-w2 part) -> each core
    processes 1088 W2 rows = 9 output tiles (NT) of U.T.
  * Streamed tensor wq: k-blocked weights, fp16, 3 k-tiles per DMA
    ([128, 3*1216]): cols 0:1088 of each k-tile = the W2 slice
    pre-transposed, cols 1088:1216 = the k-block-transposed W1 chunk
    (T1 = W1 @ xs.T is produced on device 2 k-tiles ahead of the GEMM).
  * GEMM: fp16 operands (error ~5e-4 << the 2e-2 gate), fp32 psum
    accumulation over 129 k-tiles into U.T [9 tiles x 256].
  * Scan: the gain-10 sigmoid recurrence contracts hard, so the 256-step
    chain splits into 16 chunks; chunks >=2 rerun 24 warmup steps from
    state 0.5 (numpy-validated: reproduces the exact scan to ~5e-4).
    Chunks are merged into 4 independent streams (uniform col stride 48)
    so one DVE add + one ACT sigmoid advances 4-5 chunks at once; streams
    interleave round-robin, hiding the cross-engine dependency latency.
  * Prediction: prods fw1*x per m-tile (split Pool/DVE), 8 one-hot
    "sel" matmuls place each tile's column sums at psum partition 8c+m,
    one fused max(H,0)*fw2 op, one ones-matmul -> [1,256] partials.

Hand-rolled semaphores (one wait + one inc per instruction, walrus
limit); in-order engine queues carry the rest of the ordering.
"""
import os
import sys

sys.path.insert(0, "/opt/trn_rl_repo")

import numpy as np
from contextlib import ExitStack

import concourse.bass as bass
import concourse.tile as tile
from concourse import mybir
from concourse.bass_utils import run_bass_kernel_spmd

F32 = mybir.dt.float32
F16 = mybir.dt.float16
AF = mybir.ActivationFunctionType
ALU = mybir.AluOpType

IN = 128
HID = 64
NFW = IN * HID + HID          # 8256
B = 256
K2 = 2 * NFW                  # 16512
KT = K2 // 128                # 129 contraction tiles
NCORES = 8
MT_OWN = 8                    # full 128-row W2 tiles owned per core
NT = MT_OWN + 1               # + shared 64-row tile
MSL = MT_OWN * 128 + HID      # 1088 W2 rows handled per core
WC = MSL + 128                # k-tile width (W2 cols + W1 chunk)
GAIN, SHIFT = 10.0, 0.5

T3 = 3                        # k-tiles per streamed DMA
QT = KT // T3                 # 43 triple-DMAs
NQ = 8                        # triple ring slots
LOOKQ = 7                     # triples issued ahead
NT1 = 4                       # t1 ring slots

# scan streams: chunk 0 = cols 0:32 (exact, from fw0); chunks j=2..15 =
# cols 16j:16j+16 with 24 warmup steps from 0.5.  Streams group chunks
# at uniform stride 48 so one instruction advances a whole stream.
HWRM = 24
LC0 = 32                      # chunk-0 columns (rounds 0..31)
STREAMS = [[2, 5, 8, 11, 14], [3, 6, 9, 12, 15], [4, 7, 10, 13]]
ROUNDS = HWRM + 16            # 40

_NC_CACHE = None


def _build_bass():
    nc = bass.Bass("TRN2", target_bir_lowering=False, debug=False)

    wq_d = nc.dram_tensor("wq", [QT * 128, T3 * WC], F16, kind="ExternalInput")
    cst_d = nc.dram_tensor("cst", [128, B + NT], F32, kind="ExternalInput")
    sel_d = nc.dram_tensor("sel", [128, MT_OWN * HID + 1], F16,
                           kind="ExternalInput")
    pred_d = nc.dram_tensor("pred", [1, B], F32, kind="ExternalOutput")
    dbg = bool(int(os.environ.get("KDBG", "0")))
    if dbg:
        u_dbg_d = nc.dram_tensor("u_dbg", [128, NT * B], F32,
                                 kind="ExternalOutput")
        fw_dbg_d = nc.dram_tensor("fw_dbg", [128, NT * B], F32,
                                  kind="ExternalOutput")
        q_dbg_d = nc.dram_tensor("q_dbg", [HID, B], F16,
                                 kind="ExternalOutput")

    with tile.TileContext(nc) as tc:
        with ExitStack() as ctx:
            const_pool = ctx.enter_context(tc.tile_pool(name="const", bufs=1))
            stream_pool = ctx.enter_context(tc.tile_pool(name="wcs", bufs=1))
            big_pool = ctx.enter_context(tc.tile_pool(name="big", bufs=1))

            cst = const_pool.tile([128, B + NT], F32)
            xst = cst[:, 0:B]
            fw0_t = cst[:, B:B + NT]
            sel = const_pool.tile([128, MT_OWN * HID + 1], F16)
            ones16 = sel[:, MT_OWN * HID:MT_OWN * HID + 1]
            xst_h = const_pool.tile([128, B], F16)
            bias_t = const_pool.tile([128, 1], F32)
            half_t = const_pool.tile([128, NT], F32)
            zf16 = const_pool.tile([128, 128], F16)

            wbuf = stream_pool.tile([128, NQ * T3 * WC], F16)  # stream ring
            t1r = big_pool.tile([128, NT1 * B], F16)           # T1.T ring
            u_sb = big_pool.tile([128, NT * B], F32)           # U.T slice
            fw_sb = big_pool.tile([128, NT * B], F32)          # fw history
            # per-stream add/sigmoid ping-pong + warmup state slots
            t_big = big_pool.tile([128, 2 * NT * (1 + 5 + 5 + 4)], F32)
            wsc = big_pool.tile([128, 2 * NT * (5 + 5 + 4)], F32)
            prod_h = big_pool.tile([128, MT_OWN * B], F16)     # fw1*x
            q_sb = big_pool.tile([HID, B], F16)                # relu(H)*fw2
            pred_sb = big_pool.tile([1, B], F32)

            u_r = u_sb[:].rearrange("p (m i) -> p m i", m=NT)
            fw_r = fw_sb[:].rearrange("p (m i) -> p m i", m=NT)

            def qslot(q):
                s = q % NQ
                return wbuf[:, s * T3 * WC:(s + 1) * T3 * WC]

            def wslot(k):
                s = (k // T3) % NQ
                off = (s * T3 + k % T3) * WC
                return wbuf[:, off:off + WC]

            def t1slot(j):
                s = j % NT1
                return t1r[:, s * B:(s + 1) * B]

            # sub-bank psum packing for the 9 accumulating U.T tiles
            tile_bytes = B * 4
            bankof = [m * tile_bytes // 2048 for m in range(NT)]
            m_first = [m == 0 or bankof[m] != bankof[m - 1] for m in range(NT)]
            m_last = [m == NT - 1 or bankof[m] != bankof[m + 1] for m in range(NT)]

            csem = nc.alloc_semaphore("csem")
            dsem = [nc.alloc_semaphore(f"dsem{s}") for s in range(NQ)]
            tmm_sem = nc.alloc_semaphore("tmm")
            cp_sem = nc.alloc_semaphore("cp")
            pe_sem = nc.alloc_semaphore("pe")
            sv = nc.alloc_semaphore("sv")     # DVE progress
            sa = nc.alloc_semaphore("sa")     # ACT progress
            pl = nc.alloc_semaphore("pl")     # Pool progress
            pp = nc.alloc_semaphore("pp")     # PE pred progress
            dsm = nc.alloc_semaphore("dsm")

            with tc.tile_pool(name="pt1", bufs=1, space="PSUM") as pt_pool, \
                 tc.tile_pool(name="pu", bufs=1, space="PSUM") as pu_pool:
                pt = pt_pool.tile([128, 1024], F32)
                psum_u = pu_pool.tile([128, NT * B], F32)
                pu_r = psum_u[:].rearrange("p (m i) -> p m i", m=NT)

                with tc.tile_critical():
                    svc = [0]                 # sv value tracker

                    def dve_inc(inst):
                        inst.then_inc(sv, 1)
                        svc[0] += 1
                        return svc[0]

                    # constants: two DMAs on the Pool queue, DVE preps
                    nc.gpsimd.dma_start(cst[:], cst_d[:, :]).then_inc(csem, 16)
                    nc.gpsimd.dma_start(sel[:], sel_d[:, :]).then_inc(csem, 16)
                    nc.vector.memset(bias_t[:], -GAIN * SHIFT)
                    dve_inc(nc.vector.memset(half_t[:], 0.5))
                    dve_inc(nc.vector.memset(zf16[:], 0.0))
                    cxr = nc.vector.tensor_copy(xst_h[:], xst)
                    cxr._wait_ge(csem, 16)
                    v_xr = dve_inc(cxr)

                    def dma_q(q):
                        d = nc.sync.dma_start(
                            qslot(q), wq_d[q * 128:(q + 1) * 128, :])
                        if q >= NQ:
                            d._wait_ge(pe_sem, T3 * (q - NQ) + T3)
                        d.then_inc(dsem[q % NQ], 16)

                    def t1_mm(j):
                        mm = nc.tensor.matmul(
                            pt[:, (j % 2) * 512:(j % 2) * 512 + B],
                            wslot(j)[:, MSL:WC], xst_h[:],
                            start=True, stop=True)
                        q = j // T3
                        mm._wait_ge(dsem[q % NQ], 16 * (q // NQ + 1))
                        mm.then_inc(tmm_sem, 1)

                    def t1_copy(j):
                        cp = nc.vector.tensor_copy(
                            t1slot(j), pt[:, (j % 2) * 512:(j % 2) * 512 + B])
                        cp._wait_ge(tmm_sem, j + 1)
                        cp.then_inc(cp_sem, 1)

                    # PE warmup (pulls the const prep tick into PE's clock)
                    zmm = nc.tensor.matmul(pt[:, 0:B], zf16[:], xst_h[:],
                                           start=True, stop=True)
                    zmm._wait_ge(sv, v_xr)

                    for q in range(LOOKQ):
                        dma_q(q)
                    for j in range(2):
                        t1_mm(j)
                        t1_copy(j)

                    for k in range(KT):
                        if k % T3 == 0 and k // T3 + LOOKQ < QT:
                            dma_q(k // T3 + LOOKQ)
                        for m in range(NT):
                            mm = nc.tensor.matmul(
                                psum_u[:, m * B:(m + 1) * B],
                                wslot(k)[:, m * 128:(m + 1) * 128],
                                t1slot(k),
                                start=(k == 0 and m_first[m]),
                                stop=(k == KT - 1 and m_last[m]),
                            )
                            if m == 0:
                                mm._wait_ge(cp_sem, k + 1)
                            if m == NT - 1:
                                mm.then_inc(pe_sem, 1)
                        if k + 2 < KT:
                            t1_mm(k + 2)
                            t1_copy(k + 2)

                    # gate: sel-DMA arrival folded into the DVE stream here
                    # (long after issue; everything downstream inherits it)
                    g = nc.vector.memset(t_big[0:1, 0:1], 0.0)
                    g._wait_ge(csem, 32)
                    dve_inc(g)

                    # psum U -> sbuf, split DVE/ACT to halve the stall
                    ucp_d = nc.vector.tensor_copy(
                        u_sb[:, 0:NT * B // 2], psum_u[:, 0:NT * B // 2])
                    ucp_d._wait_ge(pe_sem, KT)
                    dve_inc(ucp_d)
                    ucp_a = nc.scalar.activation(
                        u_sb[:, NT * B // 2:], psum_u[:, NT * B // 2:],
                        AF.Copy)
                    ucp_a._wait_ge(pe_sem, KT)
                    ucp_a.then_inc(sa, 1)

                    # ---- stream scan ----
                    # stream 0: chunk 0 [128, NT]; streams 1-3: merged
                    # chunks [128, NT, n] at col stride 48.
                    sdesc = []      # (i0_base, n, t_off, w_off)
                    t_off = 0
                    w_off = 0
                    sdesc.append((0, 1, t_off, None))
                    t_off += 2 * NT
                    for s in STREAMS:
                        sdesc.append((16 * s[0] - HWRM, len(s), t_off, w_off))
                        t_off += 2 * NT * len(s)
                        w_off += 2 * NT * len(s)

                    def tview(si, r):
                        i0, n, to, _ = sdesc[si]
                        off = to + (r % 2) * NT * n
                        return t_big[:, off:off + NT * n].rearrange(
                            "p (m t) -> p m t", t=n)

                    def wview(si, r):
                        i0, n, _, wo = sdesc[si]
                        off = wo + (r % 2) * NT * n
                        return wsc[:, off:off + NT * n].rearrange(
                            "p (m t) -> p m t", t=n)

                    def fview(ap, si, r):
                        i0, n, _, _ = sdesc[si]
                        i = i0 + r
                        return ap[:, :, i:i + 48 * (n - 1) + 1:48]

                    v_add = {}
                    a_idx = {}
                    a_cnt = [1]               # ucp_a took sa slot 1

                    def emit_add(si, r):
                        i0, n, _, _ = sdesc[si]
                        if r == 0:
                            prev = (fw0_t[:, :, None] if si == 0
                                    else half_t[:, :, None]).broadcast_to(
                                        (128, NT, n))
                        elif si > 0 and r - 1 < HWRM:
                            prev = wview(si, r - 1)
                        else:
                            prev = fview(fw_r, si, r - 1)
                        add = nc.vector.tensor_add(
                            tview(si, r), prev, fview(u_r, si, r))
                        if r > 0:
                            add._wait_ge(sa, a_idx[(si, r - 1)])
                        else:
                            # round-0 adds read the ACT-copied half of U
                            add._wait_ge(sa, 1)
                        v_add[(si, r)] = dve_inc(add)

                    def emit_act(si, r):
                        out = (wview(si, r) if si > 0 and r < HWRM
                               else fview(fw_r, si, r))
                        act = nc.scalar.activation(
                            out, tview(si, r), AF.Sigmoid,
                            bias=bias_t[:], scale=GAIN)
                        act._wait_ge(sv, v_add[(si, r)])
                        act.then_inc(sa, 1)
                        a_cnt[0] += 1
                        a_idx[(si, r)] = a_cnt[0]

                    for r in range(ROUNDS):
                        for si in range(4):
                            if si == 0 and r >= LC0:
                                continue
                            emit_add(si, r)
                            emit_act(si, r)
                    a_total = a_cnt[0]

                    # ---- prediction partials (all post-scan) ----
                    # prods split across DVE (odd m) and Pool (even m)
                    v_prod = {}
                    for m in (1, 3, 5, 7):
                        pr = nc.vector.tensor_mul(
                            prod_h[:, m * B:(m + 1) * B], fw_r[:, m, :], xst)
                        if m == 1:
                            pr._wait_ge(sa, a_total)
                        v_prod[m] = dve_inc(pr)
                    for i, m in enumerate((0, 2, 4, 6)):
                        pr = nc.gpsimd.tensor_mul(
                            prod_h[:, m * B:(m + 1) * B], fw_r[:, m, :], xst)
                        if m == 0:
                            pr._wait_ge(sa, a_total)
                        pr.then_inc(pl, 1)

                    # H[m,:] accumulated at psum partition 8*core+m via
                    # one-hot sel matmuls (all write the same [64,256] tile)
                    order = [1, 3, 5, 7, 0, 2, 4, 6]
                    for idx, m in enumerate(order):
                        hm = nc.tensor.matmul(
                            pt[0:HID, 0:B], sel[:, m * HID:(m + 1) * HID],
                            prod_h[:, m * B:(m + 1) * B],
                            start=(idx == 0), stop=(idx == len(order) - 1))
                        if m % 2 == 1:
                            hm._wait_ge(sv, v_prod[m])
                        else:
                            hm._wait_ge(pl, m // 2 + 1)
                        if idx == len(order) - 1:
                            hm.then_inc(pp, 1)

                    # q = max(H,0) * fw2  (fused, psum+sbuf -> sbuf fp16)
                    stt = nc.vector.scalar_tensor_tensor(
                        q_sb[:, :], pt[0:HID, 0:B], 0.0,
                        fw_r[0:HID, NT - 1, :], op0=ALU.max, op1=ALU.mult)
                    stt._wait_ge(pp, 1)
                    v_stt = dve_inc(stt)

                    pmm = nc.tensor.matmul(
                        psum_u[0:1, 0:B], ones16[0:HID, :], q_sb[:, :],
                        start=True, stop=True)
                    pmm._wait_ge(sv, v_stt)
                    pmm.then_inc(pp, 1)

                    cpd = nc.vector.tensor_copy(pred_sb[:], psum_u[0:1, 0:B])
                    cpd._wait_ge(pp, 2)
                    v_out = dve_inc(cpd)

                    dout = nc.sync.dma_start(pred_d[:, :], pred_sb[:])
                    dout._wait_ge(sv, v_out)
                    dout.then_inc(dsm, 16)
                    if dbg:
                        du = nc.sync.dma_start(u_dbg_d[:, :], u_sb[:])
                        du._wait_ge(sv, v_out)
                        du.then_inc(dsm, 16)
                        df = nc.sync.dma_start(fw_dbg_d[:, :], fw_sb[:])
                        df._wait_ge(sv, v_out)
                        df.then_inc(dsm, 16)
                        dq = nc.sync.dma_start(q_dbg_d[:, :], q_sb[:])
                        dq._wait_ge(sv, v_out)
                        dq.then_inc(dsm, 16)

    _dedupe_waits(nc)
    return nc


def _dedupe_waits(nc):
    """Collapse duplicate semaphore waits the framework occasionally emits
    (walrus allows very few sync commands per instruction)."""
    for fnn in nc.m.functions:
        for blk in fnn.blocks:
            for inst in blk.instructions:
                si = inst.sync_info
                if si is None or not si.on_wait or len(si.on_wait) < 2:
                    continue
                best = {}
                order = []
                for w in si.on_wait:
                    if w.wait_reg is not None or w.wait_mode != "sem-ge-imm":
                        key = ("raw", id(w))
                    else:
                        key = (w.sync_type, w.id, w.wait_mode)
                    if key not in best:
                        best[key] = w
                        order.append(key)
                    elif (w.wait_value or 0) > (best[key].wait_value or 0):
                        best[key] = w
                deduped = [best[k] for k in order]
                if len(deduped) != len(si.on_wait):
                    inst.sync_info = mybir.SyncInfo(
                        on_wait=deduped, on_update=si.on_update)


def _split_noops(nc):
    """Split multi-wait NoOps into single-wait chains (walrus's CTRL_NO
    struct carries very few sync commands)."""
    if getattr(nc, "_noops_split", False):
        return
    nc._noops_split = True
    split_id = [0]
    for fnn in nc.m.functions:
        for blk in fnn.blocks:
            out = []
            changed = False
            for inst in blk.instructions:
                si = inst.sync_info
                if (type(inst).__name__ == "InstNoOp" and si is not None
                        and len(si.on_wait) > 1):
                    changed = True
                    for w in si.on_wait[:-1]:
                        no = mybir.InstNoOp(
                            name=f"noop_waitsplit_{split_id[0]}",
                            text_hint="waitsplit")
                        split_id[0] += 1
                        no.engine = inst.engine
                        no.sync_info = mybir.SyncInfo(
                            on_wait=[w], on_update=[])
                        out.append(no)
                    inst.sync_info = mybir.SyncInfo(
                        on_wait=[si.on_wait[-1]], on_update=si.on_update)
                out.append(inst)
            if changed:
                blk.instructions = out


def _get_nc():
    global _NC_CACHE
    if _NC_CACHE is None:
        _NC_CACHE = _build_bass()
    return _NC_CACHE


def _make_in_maps(x, W1, W2, fw0):
    xs = np.ascontiguousarray(x[:, 0, :].astype(np.float32))       # [256, 128]
    xst = np.ascontiguousarray(xs.T)                                # [128, 256]
    W1 = np.asarray(W1, dtype=np.float32)
    W2 = np.asarray(W2, dtype=np.float32)
    fw0 = np.asarray(fw0, dtype=np.float32)

    # k-block-transposed W1: rows k*128+p, col c = W1[k*128+c, p]
    w1bt = np.ascontiguousarray(
        W1.reshape(KT, 128, IN).transpose(0, 2, 1).reshape(K2, 128))

    shared_rows = W2[MT_OWN * 128 * NCORES:, :]                     # [64, 16512]
    fw_shared = np.zeros(128, np.float32)
    fw_shared[0:HID] = fw0[MT_OWN * 128 * NCORES:]

    in_maps = []
    for c in range(NCORES):
        own = W2[c * 1024:(c + 1) * 1024, :]                        # [1024, 16512]
        w2c = np.concatenate([own, shared_rows], axis=0)            # [1088, 16512]
        wcomb = np.concatenate(
            [np.ascontiguousarray(w2c.T), w1bt], axis=1)            # [16512, 1216]
        wq = np.ascontiguousarray(
            wcomb.reshape(QT, T3, 128, WC).transpose(0, 2, 1, 3)
            .reshape(QT * 128, T3 * WC)).astype(np.float16)
        fw0_t = np.zeros((128, NT), np.float32)
        for m in range(MT_OWN):
            fw0_t[:, m] = fw0[c * 1024 + m * 128: c * 1024 + (m + 1) * 128]
        fw0_t[:, NT - 1] = fw_shared
        cst = np.zeros((128, B + NT), np.float32)
        cst[:, 0:B] = xst
        cst[:, B:B + NT] = fw0_t
        sel = np.zeros((128, MT_OWN * HID + 1), np.float16)
        for m in range(MT_OWN):
            sel[:, m * HID + MT_OWN * c + m] = 1.0
        sel[:, MT_OWN * HID] = 1.0
        in_maps.append({"wq": wq, "cst": cst, "sel": sel})
    return in_maps


def kernel(x, W1, W2, fw0, _trace=False, _tmpdir=None):
    nc = _get_nc()
    _split_noops(nc)
    in_maps = _make_in_maps(x, W1, W2, fw0)
    res = run_bass_kernel_spmd(
        nc, in_maps, core_ids=list(range(NCORES)),
        trace=_trace, tmpdir=_tmpdir,
    )
    preds = np.zeros((1, B), np.float64)
    for c in range(NCORES):
        preds += res.results[c]["pred"].astype(np.float64)
    out = preds.astype(np.float32).reshape(B, 1)
    if _trace:
        return out, res
    return out
